# revision 40
# baseline (speedup 1.0000x reference)
"""Trainium2 kernel for nn_BS_Registers_density: out = U @ rho @ U.T.

U = cos(a)*cos_mask + sin(a)*sin_mask + id_mask is the identity outside its
top-left 64x64 corner (32 disjoint 2x2 Givens blocks), so the product only
modifies the first 64 rows and first 64 columns of rho.  Each of the 8 cores
owns a 512-row slab of the output:

  - bulk pass-through  out[64:, 64:] = rho[64:, 64:]   (DRAM->DRAM DMA)
  - row update         out[0:64, :]  = B @ rho[0:64, :]        (core 0's slab)
  - col update         out[:, 0:64]  = X[:, 0:64] @ B^T        (every slab)

where B = U[0:64, 0:64] and X is the row-updated rho.  The program is
uniform across cores (SPMD): the row update uses per-core masks (real on
core 0, identity elsewhere — an exact identity product); the column update
uses the real masks everywhere.

Columns of a row-major matrix make 256-byte DMA descriptors that crawl, so
the column block travels transposed: the host packs rho[64:, 0:64]^T into
the consts tensor (contiguous load), the kernel computes
out_cols^T = B @ X^T as one matmul, stores it contiguously, and the host
transposes it back while unsharding.

Hardware constraints that shape the code:
  - every instruction encodes at most ONE semaphore wait, so each PE/DVE
    instruction depends on at most one cross-engine semaphore (DMA and ACT
    results are staged through DVE copies);
  - the kernel-tail Drain cannot carry one wait per live semaphore, so the
    patched tail below spreads them across SP no-ops;
  - only 8 HWDGE completion-sem lanes exist and lane reuse adds a second
    wait, so the program uses exactly 4 HWDGE DMAs.
"""

import numpy as np

N_CORES = 8
N_FULL = 4096
SLAB = N_FULL // N_CORES  # 512
K = 64  # size of the affected corner block

# packed consts layout (f32, [64, CW]):
#   cols    0:64   row-update cos mask (real on core 0, zero elsewhere)
#   cols   64:128  row-update sin mask (real on core 0, zero elsewhere)
#   cols  128:192  row-update id mask  (real on core 0, eye elsewhere)
#   cols  192:256  real cos mask   (column update, every core)
#   cols  256:320  real sin mask
#   cols  320:384  real id mask
#   cols  384:448  eye(64)         (PE-transpose identity)
#   col   448      theta
#   col   449      theta + pi/2
#   cols  450:4546 this core's slab rows 0:64           (row-update input)
#   cols 4546:4994 this core's slab rows 64:512, cols 0:64, TRANSPOSED
CW = 450 + N_FULL + (SLAB - K)

_CACHE = {}


def _patched_drain_and_barrier(self, tick_clock, wait_clock):
    """Kernel-tail replacement for TileContext._drain_and_barrier.

    The stock tail attaches every outstanding semaphore wait to one Drain
    instruction, but the TRN2 instruction encoding holds a single semaphore
    wait, so walrus rejects it ("Too many sync wait commands").  Spread the
    waits across one SP no-op per semaphore instead, then drain + barrier.
    """
    import re

    import bass_rust
    from concourse.vector_clock import ScopedClock

    nc = self.nc
    vals = [int(x) for x in re.findall(r"\d+", repr(tick_clock.global_clock))]
    for proc, val in enumerate(vals):
        if val <= 0:
            continue
        nop = nc.sync.nop()
        mask = bass_rust.VectorClock()
        mask.require_at_least(proc, val)
        wait_clock.add_sem_waits(nop.ins, ScopedClock({None: mask}))

    nc.sync.drain()
    nc.all_engine_barrier()
    popped = nc._tile_sem_poison_stack.pop()
    assert popped is self._sem_poison
    nc.clear_and_free_semaphores(list(self.sems.allocated().values()))
    nc.all_engine_barrier()


def _build_nc():
    import concourse.bass as bass
    import concourse.tile as tile
    from concourse import mybir

    f32 = mybir.dt.float32
    Alu = mybir.AluOpType
    Act = mybir.ActivationFunctionType

    nc = bass.Bass()
    rho = nc.dram_tensor("rho", [SLAB, N_FULL], f32, kind="ExternalInput")
    consts = nc.dram_tensor("consts", [K, CW], f32, kind="ExternalInput")
    out = nc.dram_tensor("out", [SLAB, N_FULL], f32, kind="ExternalOutput")
    # out[:, 0:64]^T, transposed back by the host during unshard
    outcolst = nc.dram_tensor("outcolst", [K, SLAB], f32, kind="ExternalOutput")

    tile.TileContext._drain_and_barrier = _patched_drain_and_barrier
    with tile.TileContext(nc) as tc:
        with (
            tc.tile_pool(name="const", bufs=1) as const_pool,
            tc.tile_pool(name="work", bufs=1) as work,
            tc.tile_pool(name="ps_row", bufs=2, space=bass.MemorySpace.PSUM) as ps_row,
            tc.tile_pool(name="ps_sm", bufs=1, space=bass.MemorySpace.PSUM) as ps_sm,
        ):
            # DMA 1 — the consts load, first on the sync (SP) ring: it
            # drains at full rate (~3us) before the bulk copy hogs HBM, so
            # the compute chain starts early.
            ct = const_pool.tile([K, CW], f32)
            nc.sync.dma_start(out=ct[:], in_=consts[:])
            # DMAs 2+3 — bulk pass-through, never touches SBUF, split across
            # both HWDGE rings so two queues drain it in parallel (each
            # queue alone tops out near ~440GB/s of bus; two reach ~680).
            # The split point balances when each queue finishes: the scalar
            # ring starts ~4us later and also carries the stores, the sync
            # ring also carries the consts load.  (A third slice on the
            # gpsimd SWDGE queue was tried and regressed — it starts late
            # and drains slowly.)
            MID = 304
            nc.scalar.dma_start(out=out[K:MID, K:N_FULL], in_=rho[K:MID, K:N_FULL])
            nc.sync.dma_start(out=out[MID:SLAB, K:N_FULL], in_=rho[MID:SLAB, K:N_FULL])

            # Absorber: one tiny matmul whose only wait is the consts-DMA
            # lane (own PSUM tag — a reused slot would add a second wait);
            # after it the PE has observed that lane, so the real matmuls
            # can read `ct` directly with just their DVE wait.
            pa = ps_sm.tile([K, K], f32, tag="abs")
            nc.tensor.matmul(pa[:], ct[:, 0:K], ct[:, 0:K], start=True, stop=True)

            # DVE copy of the small head absorbs the DMA wait for the
            # mask/eye slices used by DVE/PE below.
            ctc = const_pool.tile([K, 450], f32)
            nc.vector.tensor_copy(ctc[:], ct[:, 0:450])
            id_c = ctc[:, 384:448]
            rows_c = ct[:, 450 : 450 + N_FULL]
            colt_c = ct[:, 450 + N_FULL : CW]

            # s = sin(a); -cos(a) = sin(-(a + pi/2)), one value per partition
            acts = const_pool.tile([K, 2], f32)
            nc.scalar.activation(acts[:, 0:1], ct[:, 448:449], Act.Sin)
            nc.scalar.activation(acts[:, 1:2], ct[:, 449:450], Act.Sin, scale=-1.0)
            sc_pair = const_pool.tile([K, 2], f32)
            nc.vector.tensor_copy(sc_pair[:], acts[:])

            # B^T = sin(a)*sinm - cos(a)*cosm + idm  (cosm is antisymmetric).
            # n_row: per-core row-update masks (identity off core 0).
            # n_col: real masks — the column update applies everywhere.
            tmp = const_pool.tile([K, K], f32)
            nc.vector.scalar_tensor_tensor(tmp[:], ctc[:, 64:128], sc_pair[:, 0:1], ctc[:, 128:192], Alu.mult, Alu.add)
            n_row = const_pool.tile([K, K], f32)
            nc.vector.scalar_tensor_tensor(n_row[:], ctc[:, 0:64], sc_pair[:, 1:2], tmp[:], Alu.mult, Alu.add)
            tmp2 = const_pool.tile([K, K], f32)
            nc.vector.scalar_tensor_tensor(tmp2[:], ctc[:, 256:320], sc_pair[:, 0:1], ctc[:, 320:384], Alu.mult, Alu.add)
            n_col = const_pool.tile([K, K], f32)
            nc.vector.scalar_tensor_tensor(n_col[:], ctc[:, 192:256], sc_pair[:, 1:2], tmp2[:], Alu.mult, Alu.add)

            # Row update: xrows = B @ rho[0:64, :]  (matmul computes lhsT.T @ rhs)
            xrows = const_pool.tile([K, N_FULL], f32)
            for j in range(N_FULL // 512):
                pr = ps_row.tile([K, 512], f32)
                nc.tensor.matmul(pr[:], n_row[:], rows_c[:, j * 512 : (j + 1) * 512], start=True, stop=True)
                nc.vector.tensor_copy(xrows[:, j * 512 : (j + 1) * 512], pr[:])
            # DMAs 4+5 — store the row block except its first 64 columns,
            # split across both rings so each half drains on whichever
            # queue has finished its bulk-copy share.
            nc.scalar.dma_start(out=out[0:K, K:2048], in_=xrows[:, K:2048])
            nc.sync.dma_start(out=out[0:K, 2048:N_FULL], in_=xrows[:, 2048:N_FULL])

            # Column update, transposed: out_cols^T = B @ X^T.
            # X^T cols 0:64 = (row-updated corner)^T via PE transpose;
            # X^T cols 64:512 = host-packed rho[64:, 0:64]^T.
            pt = ps_sm.tile([K, K], f32, tag="small")
            nc.tensor.transpose(pt[:], xrows[:, 0:K], id_c[:])
            xt = work.tile([K, SLAB], f32, tag="xt")
            nc.vector.tensor_copy(xt[:, 0:K], pt[:])
            nc.vector.tensor_copy(xt[:, K:SLAB], colt_c[:])
            pco = ps_row.tile([K, SLAB], f32, tag="pco")
            nc.tensor.matmul(pco[:], n_col[:], xt[:], start=True, stop=True)
            oct_t = work.tile([K, SLAB], f32, tag="oct")
            nc.vector.tensor_copy(oct_t[:], pco[:])
            # DMA 6 — store out_cols^T contiguously (SP ring, which is
            # idle by the time this is ready)
            nc.sync.dma_start(out=outcolst[:], in_=oct_t[:])

    return nc


def _get_nc():
    if "nc" not in _CACHE:
        _CACHE["nc"] = _build_nc()
    return _CACHE["nc"]


def pack_consts(row_masks, real_masks, theta, rows, colt):
    ct = np.empty((K, CW), dtype=np.float32)
    ct[:, 0:64] = row_masks[0]
    ct[:, 64:128] = row_masks[1]
    ct[:, 128:192] = row_masks[2]
    ct[:, 192:256] = real_masks[0]
    ct[:, 256:320] = real_masks[1]
    ct[:, 320:384] = real_masks[2]
    ct[:, 384:448] = np.eye(K, dtype=np.float32)
    ct[:, 448] = theta
    ct[:, 449] = theta + np.float32(np.pi / 2)
    ct[:, 450 : 450 + N_FULL] = rows
    ct[:, 450 + N_FULL : CW] = colt
    return ct


def _in_maps(input_state, angle, cos_matrix, sin_matrix, id_matrix):
    rho = np.ascontiguousarray(np.asarray(input_state, dtype=np.float32))
    assert rho.shape == (N_FULL, N_FULL)
    theta = np.float32(np.asarray(angle))

    corner = lambda m: np.asarray(m, dtype=np.float32)[0:K, 0:K]
    real = (corner(cos_matrix), corner(sin_matrix), corner(id_matrix))
    zeros = np.zeros((K, K), dtype=np.float32)
    ident = (zeros, zeros, np.eye(K, dtype=np.float32))

    maps = []
    for c in range(N_CORES):
        slab = rho[c * SLAB : (c + 1) * SLAB]
        ct = pack_consts(real if c == 0 else ident, real, theta, slab[0:K], slab[K:, 0:K].T)
        maps.append({"rho": slab, "consts": ct})
    return maps


def _assemble(results):
    full = np.concatenate([results[c]["out"] for c in range(N_CORES)], axis=0)
    for c in range(N_CORES):
        full[c * SLAB : (c + 1) * SLAB, 0:K] = results[c]["outcolst"].T
    return full


def run(input_state, angle, cos_matrix, sin_matrix, id_matrix, **spmd_kwargs):
    from concourse.bass_utils import run_bass_kernel_spmd

    nc = _get_nc()
    maps = _in_maps(input_state, angle, cos_matrix, sin_matrix, id_matrix)
    res = run_bass_kernel_spmd(nc, maps, list(range(N_CORES)), **spmd_kwargs)
    return _assemble(res.results).astype(np.float32, copy=False), res


def kernel(input_state, angle, cos_matrix, sin_matrix, id_matrix):
    full, _ = run(input_state, angle, cos_matrix, sin_matrix, id_matrix)
    return full


# revision 43
# speedup vs baseline: 1.0690x; 1.0690x over previous
"""Trainium2 kernel for nn_BS_Registers_density: out = U @ rho @ U.T.

U = cos(a)*cos_mask + sin(a)*sin_mask + id_mask is the identity outside its
top-left 64x64 corner (32 disjoint 2x2 Givens blocks), so the product only
modifies the first 64 rows and first 64 columns of rho.  Each of the 8 cores
owns a 512-row slab of the output:

  - bulk pass-through  out[64:, 64:] = rho[64:, 64:]   (DRAM->DRAM DMA)
  - row update         out[0:64, :]  = B @ rho[0:64, :]        (core 0's slab)
  - col update         out[:, 0:64]  = X[:, 0:64] @ B^T        (every slab)

where B = U[0:64, 0:64] and X is the row-updated rho.  The program is
uniform across cores (SPMD): the row update uses per-core masks (real on
core 0, identity elsewhere — an exact identity product); the column update
uses the real masks everywhere.

Columns of a row-major matrix make 256-byte DMA descriptors that crawl, so
the column block travels transposed: the host packs rho[64:, 0:64]^T into
the consts tensor (contiguous load), the kernel computes
out_cols^T = B @ X^T as one matmul, stores it contiguously, and the host
transposes it back while unsharding.

Hardware constraints that shape the code:
  - every instruction encodes at most ONE semaphore wait, so each PE/DVE
    instruction depends on at most one cross-engine semaphore (DMA and ACT
    results are staged through DVE copies);
  - the kernel-tail Drain cannot carry one wait per live semaphore, so the
    patched tail below spreads them across SP no-ops;
  - only 8 HWDGE completion-sem lanes exist and lane reuse adds a second
    wait, so the program uses exactly 4 HWDGE DMAs.
"""

import numpy as np

N_CORES = 8
N_FULL = 4096
SLAB = N_FULL // N_CORES  # 512
K = 64  # size of the affected corner block

# packed consts layout (f32, [64, CW]):
#   cols    0:64   row-update cos mask (real on core 0, zero elsewhere)
#   cols   64:128  row-update sin mask (real on core 0, zero elsewhere)
#   cols  128:192  row-update id mask  (real on core 0, eye elsewhere)
#   cols  192:256  real cos mask   (column update, every core)
#   cols  256:320  real sin mask
#   cols  320:384  real id mask
#   cols  384:448  eye(64)         (PE-transpose identity)
#   col   448      theta
#   col   449      theta + pi/2
#   cols  450:4546 this core's slab rows 0:64           (row-update input)
#   cols 4546:4994 this core's slab rows 64:512, cols 0:64, TRANSPOSED
CW = 450 + N_FULL + (SLAB - K)

_CACHE = {}


def _patched_drain_and_barrier(self, tick_clock, wait_clock):
    """Kernel-tail replacement for TileContext._drain_and_barrier.

    The stock tail attaches every outstanding semaphore wait to one Drain
    instruction, but the TRN2 instruction encoding holds a single semaphore
    wait, so walrus rejects it ("Too many sync wait commands").  Spread the
    waits across one SP no-op per semaphore instead, then drain + barrier.
    """
    import re

    import bass_rust
    from concourse.vector_clock import ScopedClock

    nc = self.nc
    vals = [int(x) for x in re.findall(r"\d+", repr(tick_clock.global_clock))]
    for proc, val in enumerate(vals):
        if val <= 0:
            continue
        nop = nc.sync.nop()
        mask = bass_rust.VectorClock()
        mask.require_at_least(proc, val)
        wait_clock.add_sem_waits(nop.ins, ScopedClock({None: mask}))

    nc.sync.drain()
    nc.all_engine_barrier()
    popped = nc._tile_sem_poison_stack.pop()
    assert popped is self._sem_poison
    nc.clear_and_free_semaphores(list(self.sems.allocated().values()))
    nc.all_engine_barrier()


def _build_nc():
    import concourse.bass as bass
    import concourse.tile as tile
    from concourse import mybir

    f32 = mybir.dt.float32
    Alu = mybir.AluOpType
    Act = mybir.ActivationFunctionType

    nc = bass.Bass()
    rho = nc.dram_tensor("rho", [SLAB, N_FULL], f32, kind="ExternalInput")
    consts = nc.dram_tensor("consts", [K, CW], f32, kind="ExternalInput")
    out = nc.dram_tensor("out", [SLAB, N_FULL], f32, kind="ExternalOutput")
    # out[:, 0:64]^T, transposed back by the host during unshard
    outcolst = nc.dram_tensor("outcolst", [K, SLAB], f32, kind="ExternalOutput")

    tile.TileContext._drain_and_barrier = _patched_drain_and_barrier
    with tile.TileContext(nc) as tc:
        with (
            tc.tile_pool(name="const", bufs=1) as const_pool,
            tc.tile_pool(name="work", bufs=1) as work,
            tc.tile_pool(name="ps_row", bufs=2, space=bass.MemorySpace.PSUM) as ps_row,
            tc.tile_pool(name="ps_sm", bufs=1, space=bass.MemorySpace.PSUM) as ps_sm,
        ):
            # DMA 1 — the consts load, first on the sync (SP) ring: it
            # drains at full rate (~3us) before the bulk copy hogs HBM, so
            # the compute chain starts early.
            ct = const_pool.tile([K, CW], f32)
            nc.sync.dma_start(out=ct[:], in_=consts[:])
            # DMAs 2+3 — bulk pass-through, never touches SBUF, split across
            # both HWDGE rings so two queues drain it in parallel (each
            # queue alone tops out near ~440GB/s of bus; two reach ~680).
            # The split point balances when each queue finishes: the scalar
            # ring starts ~4us later and also carries the stores, the sync
            # ring also carries the consts load.  (A third slice on the
            # gpsimd SWDGE queue was tried and regressed — it starts late
            # and drains slowly.)
            MID = 272
            nc.scalar.dma_start(out=out[K:MID, K:N_FULL], in_=rho[K:MID, K:N_FULL])
            nc.sync.dma_start(out=out[MID:SLAB, K:N_FULL], in_=rho[MID:SLAB, K:N_FULL])

            # Absorber: one tiny matmul whose only wait is the consts-DMA
            # lane (own PSUM tag — a reused slot would add a second wait);
            # after it the PE has observed that lane, so the real matmuls
            # can read `ct` directly with just their DVE wait.
            pa = ps_sm.tile([K, K], f32, tag="abs")
            nc.tensor.matmul(pa[:], ct[:, 0:K], ct[:, 0:K], start=True, stop=True)

            # DVE copy of the small head absorbs the DMA wait for the
            # mask/eye slices used by DVE/PE below.
            ctc = const_pool.tile([K, 450], f32)
            nc.vector.tensor_copy(ctc[:], ct[:, 0:450])
            id_c = ctc[:, 384:448]
            rows_c = ct[:, 450 : 450 + N_FULL]
            colt_c = ct[:, 450 + N_FULL : CW]

            # s = sin(a); -cos(a) = sin(-(a + pi/2)), one value per partition
            acts = const_pool.tile([K, 2], f32)
            nc.scalar.activation(acts[:, 0:1], ct[:, 448:449], Act.Sin)
            nc.scalar.activation(acts[:, 1:2], ct[:, 449:450], Act.Sin, scale=-1.0)
            sc_pair = const_pool.tile([K, 2], f32)
            nc.vector.tensor_copy(sc_pair[:], acts[:])

            # B^T = sin(a)*sinm - cos(a)*cosm + idm  (cosm is antisymmetric).
            # n_row: per-core row-update masks (identity off core 0).
            # n_col: real masks — the column update applies everywhere.
            tmp = const_pool.tile([K, K], f32)
            nc.vector.scalar_tensor_tensor(tmp[:], ctc[:, 64:128], sc_pair[:, 0:1], ctc[:, 128:192], Alu.mult, Alu.add)
            n_row = const_pool.tile([K, K], f32)
            nc.vector.scalar_tensor_tensor(n_row[:], ctc[:, 0:64], sc_pair[:, 1:2], tmp[:], Alu.mult, Alu.add)
            tmp2 = const_pool.tile([K, K], f32)
            nc.vector.scalar_tensor_tensor(tmp2[:], ctc[:, 256:320], sc_pair[:, 0:1], ctc[:, 320:384], Alu.mult, Alu.add)
            n_col = const_pool.tile([K, K], f32)
            nc.vector.scalar_tensor_tensor(n_col[:], ctc[:, 192:256], sc_pair[:, 1:2], tmp2[:], Alu.mult, Alu.add)

            # Row update: xrows = B @ rho[0:64, :]  (matmul computes lhsT.T @ rhs)
            xrows = const_pool.tile([K, N_FULL], f32)
            for j in range(N_FULL // 512):
                pr = ps_row.tile([K, 512], f32)
                nc.tensor.matmul(pr[:], n_row[:], rows_c[:, j * 512 : (j + 1) * 512], start=True, stop=True)
                nc.vector.tensor_copy(xrows[:, j * 512 : (j + 1) * 512], pr[:])
            # DMA 4 — store the row block except its first 64 columns
            nc.scalar.dma_start(out=out[0:K, K:N_FULL], in_=xrows[:, K:N_FULL])

            # Column update, transposed: out_cols^T = B @ X^T.
            # X^T cols 0:64 = (row-updated corner)^T via PE transpose;
            # X^T cols 64:512 = host-packed rho[64:, 0:64]^T.
            pt = ps_sm.tile([K, K], f32, tag="small")
            nc.tensor.transpose(pt[:], xrows[:, 0:K], id_c[:])
            xt = work.tile([K, SLAB], f32, tag="xt")
            nc.vector.tensor_copy(xt[:, 0:K], pt[:])
            nc.vector.tensor_copy(xt[:, K:SLAB], colt_c[:])
            pco = ps_row.tile([K, SLAB], f32, tag="pco")
            nc.tensor.matmul(pco[:], n_col[:], xt[:], start=True, stop=True)
            oct_t = work.tile([K, SLAB], f32, tag="oct")
            nc.vector.tensor_copy(oct_t[:], pco[:])
            # DMA 5 — store out_cols^T contiguously
            nc.scalar.dma_start(out=outcolst[:], in_=oct_t[:])

    return nc


def _get_nc():
    if "nc" not in _CACHE:
        _CACHE["nc"] = _build_nc()
    return _CACHE["nc"]


def pack_consts(row_masks, real_masks, theta, rows, colt):
    ct = np.empty((K, CW), dtype=np.float32)
    ct[:, 0:64] = row_masks[0]
    ct[:, 64:128] = row_masks[1]
    ct[:, 128:192] = row_masks[2]
    ct[:, 192:256] = real_masks[0]
    ct[:, 256:320] = real_masks[1]
    ct[:, 320:384] = real_masks[2]
    ct[:, 384:448] = np.eye(K, dtype=np.float32)
    ct[:, 448] = theta
    ct[:, 449] = theta + np.float32(np.pi / 2)
    ct[:, 450 : 450 + N_FULL] = rows
    ct[:, 450 + N_FULL : CW] = colt
    return ct


def _in_maps(input_state, angle, cos_matrix, sin_matrix, id_matrix):
    rho = np.ascontiguousarray(np.asarray(input_state, dtype=np.float32))
    assert rho.shape == (N_FULL, N_FULL)
    theta = np.float32(np.asarray(angle))

    corner = lambda m: np.asarray(m, dtype=np.float32)[0:K, 0:K]
    real = (corner(cos_matrix), corner(sin_matrix), corner(id_matrix))
    zeros = np.zeros((K, K), dtype=np.float32)
    ident = (zeros, zeros, np.eye(K, dtype=np.float32))

    maps = []
    for c in range(N_CORES):
        slab = rho[c * SLAB : (c + 1) * SLAB]
        ct = pack_consts(real if c == 0 else ident, real, theta, slab[0:K], slab[K:, 0:K].T)
        maps.append({"rho": slab, "consts": ct})
    return maps


def _assemble(results):
    full = np.concatenate([results[c]["out"] for c in range(N_CORES)], axis=0)
    for c in range(N_CORES):
        full[c * SLAB : (c + 1) * SLAB, 0:K] = results[c]["outcolst"].T
    return full


def run(input_state, angle, cos_matrix, sin_matrix, id_matrix, **spmd_kwargs):
    from concourse.bass_utils import run_bass_kernel_spmd

    nc = _get_nc()
    maps = _in_maps(input_state, angle, cos_matrix, sin_matrix, id_matrix)
    res = run_bass_kernel_spmd(nc, maps, list(range(N_CORES)), **spmd_kwargs)
    return _assemble(res.results).astype(np.float32, copy=False), res


def kernel(input_state, angle, cos_matrix, sin_matrix, id_matrix):
    full, _ = run(input_state, angle, cos_matrix, sin_matrix, id_matrix)
    return full
